# revision 21
# baseline (speedup 1.0000x reference)
"""Trainium2 Bass kernel for nn_GCNTopK2 (GCN + TopKPooling, 64 graphs x 1024
nodes).

Graph-data-parallel over 8 NeuronCores (8 graphs/core). Aggregation runs as
dense per-graph adjacency-count matmuls in plain bf16 (counts exact in bf16;
the 2e-2 rel-err budget makes hi/lo splits unnecessary). All feature tensors
stay SBUF-resident with pool-tag slot reuse; adjacency streams from DRAM in
one batched DMA per graph-half (SWDGE trigger cost ~750ns each, so few big
DMAs beat many small ones). Top-k per graph is a 12-iteration fixed-count
threshold bisection on [8,1024] f32 score tiles; selection becomes a 0/1 mask
plus tanh-score scaling, so node order never changes and one adjacency serves
both conv layers (dropped rows are exact zeros). BatchNorm statistics use one
[128,4] f32 AllReduce per BN layer (the pre-collective rank-sync barrier hides
under conv1). Scores z = u.h are computed on the PE with per-graph
column-placed lhsT vectors accumulating into a [8,1024] PSUM tile; BN1's
affine is folded into the score weights (u*s) so scoring needs no separate
hbn tensor, and BN1 is later applied to h in place on the Scalar engine in
the shadow of the DVE bisection.

conv2's aggregation runs in fp8-e4m3 DoubleRow (counts <=16 are exact in
e4m3; quantizing hh to e4m3 for the agg term costs ~1.2% output L2, within
the 2e-2 budget) which halves its PE time; BN stats use AllGather + local
sum (cheaper floor than AllReduce); big streaming DMAs ride the idle sync
engine's HWDGE queue while weights/small rows use gpsimd SWDGE.

NOTE: tensor_tensor_reduce reliably crashes the NEFF on this stack
(NRT INTERNAL error at output fetch) — verified by bisection. Use
tensor_reduce / tensor_scalar+accum_out / activation+accum_out instead.
"""

import sys

import numpy as np

sys.path.insert(0, "/opt/trn_rl_repo")

import concourse.bacc as bacc  # noqa: E402
import concourse.tile as tile  # noqa: E402
from concourse import mybir  # noqa: E402
from concourse import bass2jax  # noqa: E402

import ml_dtypes  # noqa: E402

BF16 = ml_dtypes.bfloat16
F32 = mybir.dt.float32
BF = mybir.dt.bfloat16
U8 = mybir.dt.uint8
F8 = mybir.dt.float8e4

G = 64
NPG = 1024
DEG = 8
INF = 128
HID = 256
OUTF = 256
K1 = 512
K2 = 256
EPS = 1e-5
NCORES = 8
GPC = G // NCORES            # 8 graphs per core
NODES = GPC * NPG            # 8192 nodes per core
NCH = NODES // 512           # 16 chunks of 512 nodes
P = 128
BIG = 1.0e30
BISECT_ITERS = 12

AF = mybir.ActivationFunctionType
ALU = mybir.AluOpType
AX = mybir.AxisListType


# =========================================================================
# Device program
# =========================================================================
def _emit(ctx, tc, io, phase_limit=99):
    nc = tc.nc

    wp = ctx.enter_context(tc.tile_pool(name="wp", bufs=1))
    bigp = ctx.enter_context(tc.tile_pool(name="bigp", bufs=1))
    mstr = ctx.enter_context(tc.tile_pool(name="mstr", bufs=3))
    sml = ctx.enter_context(tc.tile_pool(name="sml", bufs=4))
    jkp = ctx.enter_context(tc.tile_pool(name="jkp", bufs=2))
    st = ctx.enter_context(tc.tile_pool(name="st", bufs=1))
    psA = ctx.enter_context(tc.tile_pool(name="psA", bufs=2, space="PSUM"))
    psD = ctx.enter_context(tc.tile_pool(name="psD", bufs=2, space="PSUM"))
    psZ = ctx.enter_context(tc.tile_pool(name="psZ", bufs=1, space="PSUM"))
    psM = ctx.enter_context(tc.tile_pool(name="psM", bufs=2, space="PSUM"))
    dpool = ctx.enter_context(tc.tile_pool(name="dpool", bufs=1, space="DRAM"))

    def dma(dst, src):
        nc.gpsimd.dma_start(out=dst, in_=src)

    # ---- weights / constants (SBUF resident) ----
    def ldw(name, shape, dt=BF):
        t = wp.tile(shape, dt, tag=name, name=name + "_sb")
        dma(t[:], io[name][:])
        return t

    wrel1 = ldw("wrel1", [P, HID])
    wroot1 = ldw("wroot1", [P, HID])
    wrel2 = ldw("wrel2", [P, 2, HID])
    wroot2 = ldw("wroot2", [P, 2, HID])
    wl = ldw("wl", [P, 4, OUTF])
    u1f = ldw("u1f", [P, 2], F32)
    u2g8 = ldw("u2g8", [P, 2, GPC, GPC])
    u1g8 = ldw("u1g8", [P, 2, GPC, GPC])
    p64 = ldw("p64", [64, 64], F32)
    g8t = ldw("g8t", [GPC, 64])
    ones_row = ldw("ones_row", [1, P])
    ones_col8 = ldw("ones_col8", [P, GPC])
    ident = ldw("identity", [P, P])
    b1 = ldw("b1", [P, 2], F32)
    b2 = ldw("b2", [P, 2], F32)
    g1c = ldw("g1c", [P, 2], F32)
    bt1c = ldw("bt1c", [P, 2], F32)
    g2c = ldw("g2c", [P, 2], F32)
    bt2c = ldw("bt2c", [P, 2], F32)
    bl_rep = ldw("bl_rep", [GPC, OUTF], F32)

    # ---- big SBUF tiles (slot reuse via shared tags) ----
    x_nm = bigp.tile([P, GPC * 8, P], BF, tag="A", name="x_nm")
    xt = bigp.tile([P, NODES], BF, tag="B", name="xt")
    nc.sync.dma_start(out=x_nm[:], in_=io["x_nm"][:])
    nc.sync.dma_start(out=xt[:], in_=io["xt"][:])
    hT = [bigp.tile([P, NODES], BF, tag=t, name=f"hT{m}")
          for m, t in ((0, "C"), (1, "D"))]
    h1T = [bigp.tile([P, NODES], BF, tag=t, name=f"h1T{m}")
           for m, t in ((0, "E"), (1, "F"))]

    # ---- DRAM tiles for collectives ----
    cc1_i = dpool.tile([P, 4], F32, tag="cc1_i", name="cc1_i")
    cc1_o = dpool.tile([NCORES, P, 4], F32, tag="cc1_o", name="cc1_o",
                       addr_space="Shared")
    cc2_i = dpool.tile([P, 4], F32, tag="cc2_i", name="cc2_i")
    cc2_o = dpool.tile([NCORES, P, 4], F32, tag="cc2_o", name="cc2_o",
                       addr_space="Shared")
    svrow1_d = dpool.tile([1, NODES], BF, tag="svrow1_d", name="svrow1_d")
    svrow2_d = dpool.tile([1, NODES], BF, tag="svrow2_d", name="svrow2_d")

    # accumulators
    s1acc = st.tile([P, 2, NCH], F32, tag="s1acc", name="s1acc")
    q1acc = st.tile([P, 2, NCH], F32, tag="q1acc", name="q1acc")
    r1max = st.tile([P, 2, NCH], F32, tag="r1max", name="r1max")
    r1sum = st.tile([P, 2, NCH], F32, tag="r1sum", name="r1sum")
    q2acc = st.tile([P, 2, NCH], F32, tag="q2acc", name="q2acc")
    r2max = st.tile([P, 2, NCH], F32, tag="r2max", name="r2max")
    r2sum = st.tile([P, 2, NCH], F32, tag="r2sum", name="r2sum")

    # ================= conv1 =================
    for g in range(GPC):
        for dh in range(2):
            nch = g * 2 + dh
            nsl = slice(nch * 512, (nch + 1) * 512)
            dsl = slice(dh * 512, (dh + 1) * 512)
            aggps = psA.tile([P, 512], F32, tag="agg", name="aggps")
            mt8 = mstr.tile([P, 8, 512], BF, tag="mt", name="mt")
            nc.sync.dma_start(out=mt8[:], in_=io["m_adj"][g, dh])
            for sc in range(8):
                nc.tensor.matmul(aggps[:], x_nm[:, g * 8 + sc, :],
                                 mt8[:, sc, :],
                                 start=(sc == 0), stop=(sc == 7))
            agg_bf = sml.tile([P, 512], BF, tag="aggbf", name="agg_bf")
            nc.vector.tensor_copy(agg_bf[:], aggps[:])
            for mch in range(2):
                msl = slice(mch * P, (mch + 1) * P)
                hps = psD.tile([P, 512], F32, tag="hps", name="hps")
                nc.tensor.matmul(hps[:], wrel1[:, msl], agg_bf[:],
                                 start=True, stop=False)
                nc.tensor.matmul(hps[:], wroot1[:, msl], xt[:, nsl],
                                 start=False, stop=True)
                # h = gelu(hps + b1); also accumulate sum for BN1
                nc.scalar.activation(
                    hT[mch][:, nsl], hps[:], AF.Gelu,
                    bias=b1[:, mch:mch + 1],
                    accum_out=s1acc[:, mch, nch:nch + 1])
                jsq = jkp.tile([P, 512], BF, tag="jsq", name="jsq")
                nc.vector.tensor_tensor(out=jsq[:], in0=hT[mch][:, nsl],
                                        in1=hT[mch][:, nsl], op=ALU.mult)
                nc.vector.tensor_reduce(q1acc[:, mch, nch:nch + 1], jsq[:],
                                        axis=AX.X, op=ALU.add)

    if phase_limit <= 1:
        return

    # ================= BN helper =================
    def bn_affine(sacc, qacc, cc_i, cc_o, count, gg, bb, tg):
        ssum = st.tile([P, 2], F32, tag=tg + "ss", name=tg + "ss")
        qsum = st.tile([P, 2], F32, tag=tg + "qs", name=tg + "qs")
        nc.vector.tensor_reduce(ssum[:], sacc[:], axis=AX.X, op=ALU.add)
        nc.vector.tensor_reduce(qsum[:], qacc[:], axis=AX.X, op=ALU.add)
        stat4 = st.tile([P, 4], F32, tag=tg + "s4", name=tg + "s4")
        nc.vector.tensor_copy(stat4[:, 0:2], ssum[:])
        nc.vector.tensor_copy(stat4[:, 2:4], qsum[:])
        dma(cc_i[:], stat4[:])
        nc.gpsimd.collective_compute(
            "AllGather", ALU.bypass, replica_groups=[list(range(NCORES))],
            ins=[cc_i[:]], outs=[cc_o[:]])
        st4g = st.tile([P, NCORES, 4], F32, tag=tg + "s4g", name=tg + "s4g")
        dma(st4g[:], cc_o[:].rearrange("r p c -> p r c"))
        st4r = st.tile([P, 4], F32, tag=tg + "s4r", name=tg + "s4r")
        nc.vector.tensor_reduce(
            st4r[:], st4g[:].rearrange("p r c -> p c r"),
            axis=AX.X, op=ALU.add)
        m = st.tile([P, 2], F32, tag=tg + "m", name=tg + "m")
        nc.vector.tensor_scalar_mul(m[:], st4r[:, 0:2], 1.0 / count)
        var = st.tile([P, 2], F32, tag=tg + "var", name=tg + "var")
        nc.vector.tensor_scalar_mul(var[:], st4r[:, 2:4], 1.0 / count)
        mm = st.tile([P, 2], F32, tag=tg + "mm", name=tg + "mm")
        nc.vector.tensor_tensor(out=mm[:], in0=m[:], in1=m[:], op=ALU.mult)
        nc.vector.tensor_tensor(out=var[:], in0=var[:], in1=mm[:],
                                op=ALU.subtract)
        nc.vector.tensor_scalar_add(var[:], var[:], EPS)
        sq = st.tile([P, 2], F32, tag=tg + "sq", name=tg + "sq")
        nc.scalar.activation(sq[:], var[:], AF.Sqrt)
        r = st.tile([P, 2], F32, tag=tg + "r", name=tg + "r")
        nc.vector.reciprocal(r[:], sq[:])
        tmp = st.tile([P, 2], F32, tag=tg + "tmp", name=tg + "tmp")
        for _ in range(2):
            nc.vector.tensor_tensor(out=tmp[:], in0=r[:], in1=r[:],
                                    op=ALU.mult)
            nc.vector.tensor_tensor(out=tmp[:], in0=tmp[:], in1=var[:],
                                    op=ALU.mult)
            nc.vector.tensor_scalar(out=tmp[:], in0=tmp[:], scalar1=-0.5,
                                    scalar2=1.5, op0=ALU.mult, op1=ALU.add)
            nc.vector.tensor_tensor(out=r[:], in0=r[:], in1=tmp[:],
                                    op=ALU.mult)
        s = st.tile([P, 2], F32, tag=tg + "s", name=tg + "s")
        nc.vector.tensor_tensor(out=s[:], in0=gg[:], in1=r[:], op=ALU.mult)
        t = st.tile([P, 2], F32, tag=tg + "t", name=tg + "t")
        nc.vector.tensor_tensor(out=t[:], in0=m[:], in1=s[:], op=ALU.mult)
        nc.vector.tensor_tensor(out=t[:], in0=bb[:], in1=t[:], op=ALU.subtract)
        return s, t

    s1t, t1t = bn_affine(s1acc, q1acc, cc1_i, cc1_o, float(G * NPG),
                         g1c, bt1c, "b1_")

    if phase_limit <= 2:
        return

    # ================= z1 scores =================
    # z1 = u1.hbn = sum_f (u1*s1)_f h_f + sum_f u1_f t1_f
    wz8 = st.tile([P, 2, GPC, GPC], BF, tag="wz8", name="wz8")
    for kc in range(2):
        nc.vector.tensor_scalar(out=wz8[:, kc], in0=u1g8[:, kc],
                                scalar1=s1t[:, kc:kc + 1], scalar2=None,
                                op0=ALU.mult)
    pb = st.tile([P, 2], BF, tag="pb", name="pb")
    ptmp = st.tile([P, 2], F32, tag="ptmp", name="ptmp")
    nc.vector.tensor_tensor(out=ptmp[:], in0=u1f[:], in1=t1t[:], op=ALU.mult)
    nc.vector.tensor_copy(pb[:], ptmp[:])
    c1ps = psM.tile([GPC, 2], F32, tag="svp", name="c1ps")
    nc.tensor.matmul(c1ps[:], ones_col8[:], pb[:], start=True, stop=True)
    c1v = st.tile([GPC, 1], F32, tag="c1v", name="c1v")
    nc.vector.tensor_reduce(c1v[:], c1ps[:], axis=AX.X, op=ALU.add)

    zps = psZ.tile([GPC, NPG], F32, tag="z", name="zps1")
    for half in range(2):
        hsl = slice(half * 512, (half + 1) * 512)
        for g in range(GPC):
            for kc in range(2):
                nc.tensor.matmul(
                    zps[:, hsl], wz8[:, kc, g, :],
                    hT[kc][:, g * NPG + half * 512: g * NPG + half * 512 + 512],
                    start=(g == 0 and kc == 0), stop=(g == GPC - 1 and kc == 1))
    zsb = st.tile([GPC, NPG], F32, tag="zsb", name="zsb")
    nc.vector.tensor_scalar(out=zsb[:], in0=zps[:], scalar1=c1v[:],
                            scalar2=None, op0=ALU.add)
    # hT <- bn1(hT) in place (ACT, overlaps bisection on DVE)
    for nch in range(NCH):
        nsl = slice(nch * 512, (nch + 1) * 512)
        for mch in range(2):
            nc.scalar.activation(hT[mch][:, nsl], hT[mch][:, nsl],
                                 AF.Identity, bias=t1t[:, mch:mch + 1],
                                 scale=s1t[:, mch:mch + 1])

    if phase_limit <= 3:
        return

    # ================= top-k threshold bisection =================
    def bisect(z, k, tg, lo_src, hi_src):
        """z: [8,1024] scores; returns (thr8 [8,1] f32, mask8 [8,1024] bf16).

        Counting runs on a [64,128] reshape (partition 8g+j) so the is_ge
        pass touches 128 elems/lane; per-graph counts come from one
        block-diag-ones matmul that also broadcasts them back to all 8
        partitions of a graph."""
        zf = st.tile([64, P], F32, tag="pk_zf", name=tg + "zf")
        dma(zf[:], z[:])
        lo8 = st.tile([GPC, 2], F32, tag="pk_lo8", name=tg + "lo8")
        nc.vector.tensor_reduce(lo8[:, 0:1], lo_src[:], axis=AX.X, op=ALU.min)
        nc.vector.tensor_scalar_add(lo8[:, 0:1], lo8[:, 0:1], -1.0)
        nc.vector.tensor_reduce(lo8[:, 1:2], hi_src[:], axis=AX.X, op=ALU.max)
        nc.vector.tensor_scalar_add(lo8[:, 1:2], lo8[:, 1:2], 1.0)
        lo8b = st.tile([GPC, 2], BF, tag="pk_lo8b", name=tg + "lo8b")
        nc.vector.tensor_copy(lo8b[:], lo8[:])
        bps = psM.tile([64, 2], F32, tag="svp", name=tg + "bps")
        nc.tensor.matmul(bps[:], g8t[:], lo8b[:], start=True, stop=True)
        lohi = st.tile([64, 2], F32, tag="pk_lohi", name=tg + "lohi")
        nc.vector.tensor_copy(lohi[:], bps[:])
        t = st.tile([64, 1], F32, tag="pk_t", name=tg + "t")
        cnt = st.tile([64, 1], F32, tag="pk_cnt", name=tg + "cnt")
        cond = st.tile([64, 1], U8, tag="pk_cond", name=tg + "cond")
        ncnd = st.tile([64, 1], U8, tag="pk_ncnd", name=tg + "ncnd")
        for _ in range(BISECT_ITERS):
            nc.vector.tensor_scalar(out=t[:], in0=lohi[:, 0:1],
                                    scalar1=lohi[:, 1:2],
                                    scalar2=0.5, op0=ALU.add, op1=ALU.mult)
            jb = jkp.tile([64, P], BF, tag="jb", name=tg + "jb")
            nc.vector.tensor_scalar(out=jb[:], in0=zf[:], scalar1=t[:],
                                    scalar2=0.0, op0=ALU.is_ge, op1=ALU.add,
                                    accum_out=cnt[:])
            cps = psM.tile([64, 1], F32, tag="svp", name=tg + "cps")
            nc.tensor.matmul(cps[:], p64[:], cnt[:], start=True, stop=True)
            nc.vector.tensor_scalar(out=cond[:], in0=cps[:], scalar1=float(k),
                                    scalar2=None, op0=ALU.is_ge)
            nc.vector.tensor_scalar(out=ncnd[:], in0=cps[:], scalar1=float(k),
                                    scalar2=None, op0=ALU.is_lt)
            nc.vector.copy_predicated(lohi[:, 0:1], cond[:], t[:])
            nc.vector.copy_predicated(lohi[:, 1:2], ncnd[:], t[:])
        mask64 = st.tile([64, P], BF, tag="pk_m64", name=tg + "m64")
        nc.vector.tensor_scalar(out=mask64[:], in0=zf[:],
                                scalar1=lohi[:, 0:1],
                                scalar2=None, op0=ALU.is_ge)
        mask8 = st.tile([GPC, NPG], BF, tag=tg + "mask8", name=tg + "mask8")
        dma(mask8[:], mask64[:])
        return mask8, mask64

    mask1, mask1_64 = bisect(zsb, K1, "p1_", zsb, zsb)
    # alive mask in node-major: [node-in-block (partition), block]
    alnps = psM.tile([P, 64], BF, tag="svp", name="alnps")
    nc.tensor.transpose(alnps[:], mask1_64[:], ident[0:64, 0:64])
    alive_nm = st.tile([P, 64], F32, tag="alive_nm", name="alive_nm")
    nc.vector.tensor_copy(alive_nm[:], alnps[:])
    mask1u = st.tile([GPC, NPG], U8, tag="mask1u", name="mask1u")
    nc.vector.tensor_scalar(out=mask1u[:], in0=mask1[:], scalar1=0.5,
                            scalar2=None, op0=ALU.is_ge)
    tb1 = st.tile([GPC, NPG], BF, tag="zx", name="tb1")
    nc.scalar.activation(tb1[:], zsb[:], AF.Tanh)
    sv1 = st.tile([GPC, NPG], BF, tag="sv", name="sv1")
    nc.vector.tensor_tensor(out=sv1[:], in0=tb1[:], in1=mask1[:], op=ALU.mult)
    # flatten per-graph rows to a DRAM row (matmul operands need base
    # partition 0/32/64, so they bounce through DRAM to partition 0)
    dma(svrow1_d[:], sv1[:])

    if phase_limit <= 4:
        return

    # ================= pool1: h1 = hbn*sv, readouts, BN2 stats ============
    for nch in range(NCH):
        nsl = slice(nch * 512, (nch + 1) * 512)
        svc = sml.tile([1, 512], BF, tag="svc", name="svc1")
        dma(svc[:], svrow1_d[0:1, nsl])
        svps = psM.tile([P, 512], F32, tag="svp", name="svps")
        nc.tensor.matmul(svps[:], ones_row[:], svc[0:1, :],
                         start=True, stop=True)
        for mch in range(2):
            nc.vector.tensor_tensor(out=h1T[mch][:, nsl],
                                    in0=hT[mch][:, nsl],
                                    in1=svps[:], op=ALU.mult)
            jsm1 = jkp.tile([P, 512], BF, tag="jsq", name="jsm1")
            nc.scalar.activation(jsm1[:], h1T[mch][:, nsl], AF.Identity,
                                 accum_out=r1sum[:, mch, nch:nch + 1])
            jsq = jkp.tile([P, 512], BF, tag="jsq", name="jsq2")
            nc.scalar.activation(jsq[:], h1T[mch][:, nsl], AF.Square,
                                 accum_out=q2acc[:, mch, nch:nch + 1])
            nc.vector.tensor_reduce(r1max[:, mch, nch:nch + 1],
                                    h1T[mch][:, nsl], axis=AX.X, op=ALU.max)

    if phase_limit <= 5:
        return

    s2t, t2t = bn_affine(r1sum, q2acc, cc2_i, cc2_o, float(G * K1),
                         g2c, bt2c, "b2_")

    # ================= hh = gelu(bn2(h1)) * alive; transpose ==============
    hhT = [bigp.tile([P, NODES], BF, tag=t, name=f"hhT{m}")
           for m, t in ((0, "C"), (1, "D"))]
    for nch in range(NCH):
        nsl = slice(nch * 512, (nch + 1) * 512)
        for mch in range(2):
            # hhT stays unmasked: its dead rows only reach h2 columns and
            # z2 entries that pool2/readout mask anyway
            nc.scalar.activation(hhT[mch][:, nsl], h1T[mch][:, nsl], AF.Gelu,
                                 bias=t2t[:, mch:mch + 1],
                                 scale=s2t[:, mch:mch + 1])

    hh_nm = bigp.tile([P, GPC * 8, HID], F8, tag="A", name="hh_nm")
    for mch in range(2):
        for nb4 in range(NCH):
            tp = psM.tile([P, 512], BF, tag="svp", name="tp")
            for q in range(4):
                nc.tensor.transpose(
                    tp[:, q * P:(q + 1) * P],
                    hhT[mch][:, (nb4 * 4 + q) * P:(nb4 * 4 + q + 1) * P],
                    ident[:])
            for q in range(4):
                nb = nb4 * 4 + q
                nc.vector.tensor_scalar(
                    out=hh_nm[:, nb, mch * P:(mch + 1) * P],
                    in0=tp[:, q * P:(q + 1) * P],
                    scalar1=alive_nm[:, nb:nb + 1],
                    scalar2=None, op0=ALU.mult)

    if phase_limit <= 6:
        return

    # ================= conv2 + z2 =================
    h2T = [bigp.tile([P, NODES], BF, tag=t, name=f"h2T{m}")
           for m, t in ((0, "B"), (1, "H"))]
    for dh in range(2):
        dsl = slice(dh * 512, (dh + 1) * 512)
        for g in range(GPC):
            nch = g * 2 + dh
            nsl = slice(nch * 512, (nch + 1) * 512)
            a2ps = [psA.tile([P, 512], F32, tag="agg", name=f"a2ps{fc}")
                    for fc in range(2)]
            mt8 = mstr.tile([P, 8, 512], F8, tag="mt8", name="mt2")
            nc.sync.dma_start(out=mt8[:], in_=io["m_adj8"][g, dh])
            for q in range(4):
                for fc in range(2):
                    nc.tensor.matmul(
                        a2ps[fc][:],
                        hh_nm[:, g * 8 + 2 * q:g * 8 + 2 * q + 2,
                              fc * P:(fc + 1) * P],
                        mt8[:, 2 * q:2 * q + 2, :],
                        start=(q == 0), stop=(q == 3),
                        perf_mode=mybir.MatmulPerfMode.DoubleRow)
            a2sb = sml.tile([P, 2, 512], BF, tag="a2sb", name="a2sb")
            for fc in range(2):
                nc.vector.tensor_copy(a2sb[:, fc, :], a2ps[fc][:])
            for mch in range(2):
                msl = slice(mch * P, (mch + 1) * P)
                h2ps = psD.tile([P, 512], F32, tag="hps", name="h2ps")
                for kc in range(2):
                    nc.tensor.matmul(h2ps[:], wrel2[:, kc, msl],
                                     a2sb[:, kc, :],
                                     start=(kc == 0), stop=False)
                    nc.tensor.matmul(h2ps[:], wroot2[:, kc, msl],
                                     hhT[kc][:, nsl],
                                     start=False, stop=(kc == 1))
                nc.scalar.activation(h2T[mch][:, nsl], h2ps[:], AF.Identity,
                                     bias=b2[:, mch:mch + 1])

    if phase_limit <= 7:
        return

    # ================= pool2 =================
    zps2 = psZ.tile([GPC, NPG], F32, tag="z", name="zps2")
    for half in range(2):
        hsl = slice(half * 512, (half + 1) * 512)
        for g in range(GPC):
            for kc in range(2):
                nc.tensor.matmul(
                    zps2[:, hsl], u2g8[:, kc, g, :],
                    h2T[kc][:, g * NPG + half * 512: g * NPG + half * 512
                            + 512],
                    start=(g == 0 and kc == 0),
                    stop=(g == GPC - 1 and kc == 1))
    zsb2 = st.tile([GPC, NPG], F32, tag="zsb", name="zsb2")
    nc.vector.tensor_copy(zsb2[:], zps2[:])
    z2m = st.tile([GPC, NPG], F32, tag="z2m", name="z2m")
    nc.vector.memset(z2m[:], -BIG)
    nc.vector.copy_predicated(z2m[:], mask1u[:], zsb2[:])
    zpos = st.tile([GPC, NPG], F32, tag="zx", name="zpos")
    nc.vector.memset(zpos[:], BIG)
    nc.vector.copy_predicated(zpos[:], mask1u[:], zsb2[:])
    mask2, _m264 = bisect(z2m, K2, "p2_", zpos, z2m)
    tb2 = st.tile([GPC, NPG], BF, tag="sv", name="tb2")
    nc.scalar.activation(tb2[:], zsb2[:], AF.Tanh)
    sv2 = st.tile([GPC, NPG], BF, tag="sv2", name="sv2")
    nc.vector.tensor_tensor(out=sv2[:], in0=tb2[:], in1=mask2[:], op=ALU.mult)
    dma(svrow2_d[:], sv2[:])

    if phase_limit <= 8:
        return

    # ================= readout2 =================
    for nch in range(NCH):
        nsl = slice(nch * 512, (nch + 1) * 512)
        svc2 = sml.tile([1, 512], BF, tag="svc", name="svc2")
        dma(svc2[:], svrow2_d[0:1, nsl])
        svps2 = psM.tile([P, 512], F32, tag="svp", name="svps2")
        nc.tensor.matmul(svps2[:], ones_row[:], svc2[0:1, :],
                         start=True, stop=True)
        for mch in range(2):
            prod = jkp.tile([P, 512], F32, tag="prod", name="prod")
            nc.vector.tensor_tensor(out=prod[:], in0=h2T[mch][:, nsl],
                                    in1=svps2[:], op=ALU.mult)
            jsm = jkp.tile([P, 512], BF, tag="jsq", name="jsm")
            nc.scalar.activation(jsm[:], prod[:], AF.Identity,
                                 accum_out=r2sum[:, mch, nch:nch + 1])
            nc.vector.tensor_reduce(r2max[:, mch, nch:nch + 1], prod[:],
                                    axis=AX.X, op=ALU.max)

    # ================= final linear =================
    xc = st.tile([P, 4, GPC], F32, tag="xc", name="xc")
    tmpa = st.tile([P, GPC], F32, tag="tmpa", name="tmpa")
    tmpb = st.tile([P, GPC], F32, tag="tmpb", name="tmpb")
    for mch in range(2):
        nc.vector.tensor_reduce(
            tmpa[:], r1max[:, mch, :].rearrange("p (g d) -> p g d", d=2),
            axis=AX.X, op=ALU.max)
        nc.vector.tensor_reduce(
            tmpb[:], r2max[:, mch, :].rearrange("p (g d) -> p g d", d=2),
            axis=AX.X, op=ALU.max)
        nc.vector.tensor_tensor(out=xc[:, mch, :], in0=tmpa[:], in1=tmpb[:],
                                op=ALU.add)
        nc.vector.tensor_reduce(
            tmpa[:], r1sum[:, mch, :].rearrange("p (g d) -> p g d", d=2),
            axis=AX.X, op=ALU.add)
        nc.vector.tensor_scalar_mul(tmpa[:], tmpa[:], 1.0 / K1)
        nc.vector.tensor_reduce(
            tmpb[:], r2sum[:, mch, :].rearrange("p (g d) -> p g d", d=2),
            axis=AX.X, op=ALU.add)
        nc.vector.tensor_scalar_mul(tmpb[:], tmpb[:], 1.0 / K2)
        nc.vector.tensor_tensor(out=xc[:, 2 + mch, :], in0=tmpa[:],
                                in1=tmpb[:], op=ALU.add)
    xch = st.tile([P, 4, GPC], BF, tag="xch", name="xch")
    nc.vector.tensor_copy(xch[:], xc[:])
    ops_f = psD.tile([GPC, OUTF], F32, tag="hps", name="ops_f")
    for kc in range(4):
        nc.tensor.matmul(ops_f[:], xch[:, kc, :], wl[:, kc, :],
                         start=(kc == 0), stop=(kc == 3))
    out_sb = st.tile([GPC, OUTF], F32, tag="out_sb", name="out_sb")
    nc.vector.tensor_tensor(out=out_sb[:], in0=ops_f[:], in1=bl_rep[:],
                            op=ALU.add)
    dma(io["out"][:], out_sb[:])

    # small debug outputs
    dbg = st.tile([P, 8], F32, tag="dbg", name="dbg")
    nc.vector.tensor_copy(dbg[:, 0:2], s1t[:])
    nc.vector.tensor_copy(dbg[:, 2:4], t1t[:])
    nc.vector.tensor_copy(dbg[:, 4:6], s2t[:])
    nc.vector.tensor_copy(dbg[:, 6:8], t2t[:])
    dma(io["dbg"][:], dbg[:])
    thrs = st.tile([GPC, 2], F32, tag="thrs", name="thrs")
    nc.vector.memset(thrs[:], 0.0)
    dma(io["thrs"][:], thrs[:])


# =========================================================================
# Build
# =========================================================================
_CACHE = {}


def _build_program():
    import os
    phase_limit = int(os.environ.get("KPHASE", "99"))
    if "nc" in _CACHE:
        return _CACHE["nc"], _CACHE["io"]
    nc = bacc.Bacc("TRN2", target_bir_lowering=False, debug=False,
                   num_devices=NCORES)
    io = {}

    def din(name, shape, dt=BF):
        io[name] = nc.dram_tensor(name, shape, dt, kind="ExternalInput").ap()

    din("m_adj", [GPC, 2, P, 8, 512])
    din("m_adj8", [GPC, 2, P, 8, 512], F8)
    din("x_nm", [P, GPC * 8, P])
    din("xt", [P, NODES])
    din("wrel1", [P, HID])
    din("wroot1", [P, HID])
    din("wrel2", [P, 2, HID])
    din("wroot2", [P, 2, HID])
    din("wl", [P, 4, OUTF])
    din("u1f", [P, 2], F32)
    din("u2g8", [P, 2, GPC, GPC])
    din("u1g8", [P, 2, GPC, GPC])
    din("p64", [64, 64], F32)
    din("g8t", [GPC, 64])
    din("ones_row", [1, P])
    din("ones_col8", [P, GPC])
    din("identity", [P, P])
    din("b1", [P, 2], F32)
    din("b2", [P, 2], F32)
    din("g1c", [P, 2], F32)
    din("bt1c", [P, 2], F32)
    din("g2c", [P, 2], F32)
    din("bt2c", [P, 2], F32)
    din("bl_rep", [GPC, OUTF], F32)
    io["out"] = nc.dram_tensor("out", [GPC, OUTF], F32,
                               kind="ExternalOutput").ap()
    io["dbg"] = nc.dram_tensor("dbg", [P, 8], F32, kind="ExternalOutput").ap()
    io["thrs"] = nc.dram_tensor("thrs", [GPC, 2], F32,
                                kind="ExternalOutput").ap()

    from contextlib import ExitStack
    with tile.TileContext(nc) as tc:
        ctx = ExitStack()
        with ctx:
            _emit(ctx, tc, io, phase_limit)
    nc.compile()
    _CACHE["nc"] = nc
    _CACHE["io"] = io
    return nc, io


# =========================================================================
# Host-side input prep
# =========================================================================
def _chunk2(w):
    return np.ascontiguousarray(
        np.asarray(w, np.float32).reshape(2, 128, -1).transpose(1, 0, 2))


def _colplace(v):
    # v: [256] -> [128, 2, GPC, GPC] with chunk kc of v in column g (row g)
    vc = np.asarray(v, np.float32).reshape(2, 128).T  # [128, 2]
    out = np.zeros((128, 2, GPC, GPC), np.float32)
    for g in range(GPC):
        out[:, :, g, g] = vc
    return out.astype(BF16)


def make_in_maps(inputs):
    x = np.asarray(inputs["x"], np.float32)
    src = np.asarray(inputs["src"], np.int64)
    dst = np.asarray(inputs["dst"], np.int64)

    W_rel1 = np.asarray(inputs["W_rel1"], np.float32)
    b_rel1 = np.asarray(inputs["b_rel1"], np.float32)
    W_root1 = np.asarray(inputs["W_root1"], np.float32)
    g1 = np.asarray(inputs["g1"], np.float32)
    bt1 = np.asarray(inputs["bt1"], np.float32)
    p1 = np.asarray(inputs["p1"], np.float32)
    g2 = np.asarray(inputs["g2"], np.float32)
    bt2 = np.asarray(inputs["bt2"], np.float32)
    W_rel2 = np.asarray(inputs["W_rel2"], np.float32)
    b_rel2 = np.asarray(inputs["b_rel2"], np.float32)
    W_root2 = np.asarray(inputs["W_root2"], np.float32)
    p2 = np.asarray(inputs["p2"], np.float32)
    Wl = np.asarray(inputs["Wl"], np.float32)
    bl = np.asarray(inputs["bl"], np.float32)

    u1 = (p1 / np.float32(np.linalg.norm(p1))).astype(np.float32)
    u2 = (p2 / np.float32(np.linalg.norm(p2))).astype(np.float32)

    sh = {
        "wrel1": W_rel1.astype(BF16),
        "wroot1": W_root1.astype(BF16),
        "wrel2": _chunk2(W_rel2).astype(BF16),
        "wroot2": _chunk2(W_root2).astype(BF16),
        "wl": np.ascontiguousarray(
            Wl.reshape(4, 128, OUTF).transpose(1, 0, 2)).astype(BF16),
        "u1f": np.ascontiguousarray(u1.reshape(2, 128).T).astype(np.float32),
        "u2g8": _colplace(u2),
        "u1g8": _colplace(u1),
        "p64": np.kron(np.eye(8, dtype=np.float32),
                       np.ones((8, 8), np.float32)),
        "g8t": np.kron(np.eye(8, dtype=BF16).astype(np.float32),
                       np.ones((1, 8), np.float32)).astype(BF16),
        "ones_row": np.ones((1, P), BF16),
        "ones_col8": np.ones((P, GPC), BF16),
        "identity": np.eye(P, dtype=BF16),
        "bl_rep": np.broadcast_to(bl, (GPC, OUTF)).astype(np.float32).copy(),
    }
    for nm, v in (("b1", b_rel1), ("b2", b_rel2), ("g1c", g1),
                  ("bt1c", bt1), ("g2c", g2), ("bt2c", bt2)):
        sh[nm] = np.ascontiguousarray(
            v.reshape(2, 128).T).astype(np.float32)

    assert np.all(src // NPG == dst // NPG), "edges must be graph-local"
    in_maps = []
    for c in range(NCORES):
        xs = x[c * NODES:(c + 1) * NODES]
        m = dict(sh)
        madj = np.zeros((GPC, NPG, NPG), np.float32)
        for gi in range(GPC):
            gg = c * GPC + gi
            e0, e1 = gg * NPG * DEG, (gg + 1) * NPG * DEG
            s_loc = src[e0:e1] - gg * NPG
            d_loc = dst[e0:e1] - gg * NPG
            cnts = np.bincount(s_loc * NPG + d_loc, minlength=NPG * NPG)
            assert cnts.max() <= 256
            madj[gi] = cnts.reshape(NPG, NPG)
        madj_r = np.ascontiguousarray(
            madj.reshape(GPC, 8, P, 2, 512).transpose(0, 3, 2, 1, 4))
        m["m_adj"] = madj_r.astype(BF16)
        m["m_adj8"] = madj_r.astype(mybir.dt.np(F8))
        assert cnts.max() <= 16
        xb = xs.astype(BF16)
        m["x_nm"] = np.ascontiguousarray(
            xb.reshape(GPC * 8, P, P).transpose(1, 0, 2))
        m["xt"] = np.ascontiguousarray(xb.T)
        in_maps.append(m)
    return in_maps


def kernel(**inputs):
    in_maps = make_in_maps(inputs)
    nc, io = _build_program()
    res = bass2jax.run_bass_via_pjrt(nc, in_maps, n_cores=NCORES)
    out = np.concatenate([res[c]["out"] for c in range(NCORES)], axis=0)
    return out.astype(np.float32)


if __name__ == "__main__":
    nc, io = _build_program()
    print("program built OK")
